# revision 1
# baseline (speedup 1.0000x reference)
"""CorrelationLayer1D Trainium2 kernel.

out[b,d,h,w] = sum_c x1[b,c,h,w] * x2[b,c,h,w-80+d]  (zero where index < 0)
B=8, C=128, H=160, W=320, D=81 (MAX_DISP=40, pad=80).

Sharding: data-parallel over batch, one batch element per NeuronCore (8 cores).

Per-core algorithm (all on device):
  For each h and each 64-wide w-chunk k, the TensorEngine computes the local
  Gram rectangle  q[m',n'] = sum_c x1[c,64k+m'] * x2pad[c,64k+n']  where
  x2pad is x2 left-padded with 80 zero columns.  The output band is the
  diagonals  out[d, 64k+m'] = q[m', m'+d], d in [0,81).
  Diagonal extraction cannot be done by compute engines (no per-partition
  offsets) nor by SBUF-side DMA APs (DGE resets the flat-step remainder at
  descriptor-run boundaries), but DRAM-side DMA APs with arbitrary outer
  strides and contiguous inner runs DO work.  So: bounce q to DRAM, re-load
  with a diagonal AP giving S[w-part, d-free], transpose on the PE via an
  identity matmul to T[d-part, w-free], and store with w contiguous.
"""

import numpy as np

B, C, H, W = 8, 128, 160, 320
D = 81
PAD = 80
MK = 64          # w-chunk width (matmul M)
NK = MK + PAD    # gram rectangle width (144)
NCHUNK = W // MK # 5
NH = 8           # h-group size
NGROUP = H // NH # 20
WP = W + PAD     # padded x2 row width (400)

_CACHE = {}


def _build_nc(repeat=1):
    import concourse.bass as bass
    import concourse.bacc as bacc
    import concourse.mybir as mybir
    from concourse import tile

    f32 = mybir.dt.float32
    nc = bacc.Bacc()

    x1 = nc.dram_tensor("x1", [C, H, W], f32, kind="ExternalInput")
    x2 = nc.dram_tensor("x2", [C, H, W], f32, kind="ExternalInput")
    ident = nc.dram_tensor("ident", [128, 128], f32, kind="ExternalInput")
    out = nc.dram_tensor("out", [D, H, W], f32, kind="ExternalOutput")

    with tile.TileContext(nc) as tc:
        with (
            tc.tile_pool(name="inpool", bufs=2) as inpool,
            tc.tile_pool(name="qpool", bufs=3) as qpool,
            tc.tile_pool(name="spool", bufs=3) as spool,
            tc.tile_pool(name="tpool", bufs=2) as tpool,
            tc.tile_pool(name="const", bufs=1) as constpool,
            tc.tile_pool(name="psq", bufs=4, space=bass.MemorySpace.PSUM) as psq,
            tc.tile_pool(name="pst", bufs=4, space=bass.MemorySpace.PSUM) as pst,
            tc.tile_pool(name="qdram", bufs=4, space="DRAM") as qdram,
        ):
            id_t = constpool.tile([128, 128], f32)
            nc.sync.dma_start(id_t[:, :], ident[:, :])

            for g in range(NGROUP * repeat):
                g = g % NGROUP
                h0 = g * NH
                # ---- load inputs for this h-group ----
                x1_t = inpool.tile([C, NH, W], f32, tag="x1t")
                nc.sync.dma_start(x1_t[:, :, :], x1[:, h0 : h0 + NH, :])
                # x2 goes into a padded layout: [C, NH, WP], first PAD cols zero
                x2_t = inpool.tile([C, NH, WP], f32, tag="x2t")
                nc.vector.memset(x2_t[:, :, 0:PAD], 0.0)
                nc.sync.dma_start(x2_t[:, :, PAD:WP], x2[:, h0 : h0 + NH, :])

                t_t = tpool.tile([D, NH, W], f32, tag="t")
                for k in range(NCHUNK):
                    # ---- gram rectangles for all h in group ----
                    q_t = qpool.tile([MK, NH, NK], f32, tag="q")
                    for hh in range(NH):
                        q_ps = psq.tile([MK, NK], f32, tag="qps")
                        nc.tensor.matmul(
                            q_ps[:, :],
                            x1_t[:, hh, k * MK : k * MK + MK],
                            x2_t[:, hh, k * MK : k * MK + NK],
                        )
                        nc.vector.tensor_copy(q_t[:, hh, :], q_ps[:, :])
                    # ---- bounce to DRAM ----
                    q_d = qdram.tile([MK, NH, NK], f32, tag="qd")
                    nc.sync.dma_start(q_d[:, :, :], q_t[:, :, :])
                    # ---- diagonal re-load: S[m', hh, e] = q_d[m', hh, m'+e] ----
                    s_t = spool.tile([MK, NH, D], f32, tag="s")
                    diag_src = bass.AP(
                        q_d.tensor,
                        q_d.offset,
                        [[NH * NK + 1, MK], [NK, NH], [1, D]],
                    )
                    nc.sync.dma_start(s_t[:, :, :], diag_src)
                    # ---- transpose S -> T via identity matmul, stash in sb ----
                    for hh in range(NH):
                        t_ps = pst.tile([D, MK], f32, tag="tps")
                        nc.tensor.matmul(
                            t_ps[:, :],
                            s_t[:, hh, :],
                            id_t[0:MK, 0:MK],
                        )
                        nc.vector.tensor_copy(
                            t_t[:, hh, k * MK : k * MK + MK], t_ps[:, :]
                        )
                # ---- store the whole h-group ----
                nc.sync.dma_start(out[:, h0 : h0 + NH, :], t_t[:, :, :])

    nc.compile()
    return nc


def _get_nc():
    if "nc" not in _CACHE:
        _CACHE["nc"] = _build_nc()
    return _CACHE["nc"]


def kernel(x_1: np.ndarray, x_2: np.ndarray) -> np.ndarray:
    from concourse.bass_utils import run_bass_kernel_spmd

    nc = _get_nc()
    x_1 = np.ascontiguousarray(x_1, dtype=np.float32)
    x_2 = np.ascontiguousarray(x_2, dtype=np.float32)
    ident = np.eye(128, dtype=np.float32)
    in_maps = [
        {"x1": x_1[b], "x2": x_2[b], "ident": ident} for b in range(B)
    ]
    res = run_bass_kernel_spmd(nc, in_maps, list(range(B)))
    return np.stack([res.results[b]["out"] for b in range(B)], axis=0)



# revision 5
# speedup vs baseline: 1.2788x; 1.2788x over previous
"""CorrelationLayer1D Trainium2 kernel (optimized).

out[b,d,h,w] = sum_c x1[b,c,h,w] * x2[b,c,h,w-80+d]  (zero where index < 0)
B=8, C=128, H=160, W=320, D=81 (MAX_DISP=40, pad=80).

Sharding: data-parallel over batch, one batch element per NeuronCore.

Key optimizations over the fp32 baseline:
  - bf16 inputs (host-cast) -> 4x TensorEngine rate + half the input DMA.
  - Two h-rows per gram matmul (M=128 full PE width, N=288): lhsT is
    x1[:, hh:hh+2, wchunk] (128 cols), rhs is x2pad[:, hh:hh+2, window]
    (288 cols); the two valid quadrants [0:64, 0:144] and [64:128,
    144:288] are copied to SBUF (bf16), halving matmul count.
  - The whole h-group's gram band is staged in one SBUF tile and written
    to DRAM with ONE large contiguous DMA (1.47 MB), then the diagonals
    S[m, d] = q[m, m+d] are re-loaded with two 4-dim diagonal APs
    (DRAM-side APs support the per-partition +1 shear stride).
  - PE transposes S [128, 81] -> T [81, 128] via identity matmul (two
    h-rows per transpose), output copied to a group-sized fp32 tile,
    stored with one DMA per group.
  - DMA queue spreading: loads + output stores issue from SP (HWDGE
    queue qSP), bounce writes + diagonal reads from ACT (qAct).
  - PSUM->SBUF copies spread across DVE / GPSIMD / ACT.
"""

import numpy as np

B, C, H, W = 8, 128, 160, 320
D = 81
PAD = 80
MK = 64           # w-chunk width per h-row
NK = MK + PAD     # gram window width (144)
NCH = W // MK     # 5 chunks
NH = 16           # h-group size
NPAIR = NH // 2   # 8 h-row pairs per group
NG = H // NH      # 10 groups
WP = W + PAD      # padded x2 row width (400)
QF = NPAIR * NCH * NK  # q_d free elems per partition (5760)

_CACHE = {}


def _build_nc():
    import concourse.bass as bass
    import concourse.bacc as bacc
    import concourse.mybir as mybir
    from concourse import tile

    f32 = mybir.dt.float32
    bf16 = mybir.dt.bfloat16
    nc = bacc.Bacc()

    x1 = nc.dram_tensor("x1", [C, H, W], bf16, kind="ExternalInput")
    x2 = nc.dram_tensor("x2", [C, H, W], bf16, kind="ExternalInput")
    ident = nc.dram_tensor("ident", [128, 128], bf16, kind="ExternalInput")
    out = nc.dram_tensor("out", [D, H, W], f32, kind="ExternalOutput")

    with tile.TileContext(nc) as tc:
        with (
            tc.tile_pool(name="inpool", bufs=2) as inpool,
            tc.tile_pool(name="qpool", bufs=3) as qpool,
            tc.tile_pool(name="spool", bufs=3) as spool,
            tc.tile_pool(name="tpool", bufs=2) as tpool,
            tc.tile_pool(name="const", bufs=1) as constpool,
            tc.tile_pool(name="psq", bufs=4, space=bass.MemorySpace.PSUM) as psq,
            tc.tile_pool(name="pst", bufs=4, space=bass.MemorySpace.PSUM) as pst,
            tc.tile_pool(name="qdram", bufs=2, space="DRAM") as qdram,
        ):
            id_t = constpool.tile([128, 128], bf16)
            nc.sync.dma_start(id_t[:, :], ident[:, :])

            for g in range(NG):
                h0 = g * NH
                # ---- load inputs for this h-group ----
                x1_t = inpool.tile([C, NH, W], bf16, tag="x1t")
                nc.sync.dma_start(x1_t[:, :, :], x1[:, h0 : h0 + NH, :])
                x2_t = inpool.tile([C, NH, WP], bf16, tag="x2t")
                nc.vector.memset(x2_t[:, :, 0:PAD], 0.0)
                nc.sync.dma_start(x2_t[:, :, PAD:WP], x2[:, h0 : h0 + NH, :])

                # ---- gram band for the whole group ----
                # q_t[p, pair, k, n]: partitions 0-63 hold even h-rows
                # (m = p), 64-127 odd h-rows (m = p-64).
                q_t = qpool.tile([128, NPAIR, NCH, NK], bf16, tag="q")
                for p_ in range(NPAIR):
                    hh = 2 * p_
                    for k in range(NCH):
                        q_ps = psq.tile([128, NK], f32, tag="qps")
                        # Two col-tiled matmuls run concurrently on the
                        # PE's column halves, filling both partition
                        # halves of one PSUM tile.
                        nc.tensor.matmul(
                            q_ps[0:64, :],
                            x1_t[:, hh, k * MK : k * MK + MK],
                            x2_t[:, hh, k * MK : k * MK + NK],
                            tile_position=(0, 0),
                        )
                        nc.tensor.matmul(
                            q_ps[64:128, :],
                            x1_t[:, hh + 1, k * MK : k * MK + MK],
                            x2_t[:, hh + 1, k * MK : k * MK + NK],
                            tile_position=(0, 64),
                        )
                        if (p_ + k) % 2 == 0:
                            nc.vector.tensor_copy(q_t[:, p_, k, :], q_ps[:, :])
                        else:
                            nc.scalar.copy(q_t[:, p_, k, :], q_ps[:, :])
                # ---- one big bounce write ----
                q_d = qdram.tile([128, NPAIR, NCH, NK], bf16, tag="qd")
                nc.scalar.dma_start(q_d[:, :, :, :], q_t[:, :, :, :])
                # ---- diagonal re-load: S[p, pair, k, d] = q_d[p, pair, k, m+d]
                s_t = spool.tile([128, NPAIR, NCH, D], bf16, tag="s")
                diag_a = bass.AP(
                    q_d.tensor,
                    q_d.offset,
                    [[QF + 1, 64], [NCH * NK, NPAIR], [NK, NCH], [1, D]],
                )
                nc.scalar.dma_start(s_t[0:64, :, :, :], diag_a)
                diag_b = bass.AP(
                    q_d.tensor,
                    q_d.offset + 64 * QF,
                    [[QF + 1, 64], [NCH * NK, NPAIR], [NK, NCH], [1, D]],
                )
                nc.scalar.dma_start(s_t[64:128, :, :, :], diag_b)
                # ---- transpose S -> T via identity matmul ----
                t_t = tpool.tile([D, NH, W], f32, tag="t")
                for p_ in range(NPAIR):
                    for k in range(NCH):
                        t_ps = pst.tile([D, 128], f32, tag="tps")
                        nc.tensor.matmul(
                            t_ps[:, :],
                            s_t[:, p_, k, :],
                            id_t[:, :],
                        )
                        if (p_ + k) % 2 == 0:
                            nc.scalar.copy(
                                t_t[
                                    :,
                                    2 * p_ : 2 * p_ + 2,
                                    k * MK : k * MK + MK,
                                ],
                                t_ps[:, :],
                            )
                        else:
                            nc.vector.tensor_copy(
                                t_t[
                                    :,
                                    2 * p_ : 2 * p_ + 2,
                                    k * MK : k * MK + MK,
                                ],
                                t_ps[:, :],
                            )
                # ---- store the whole h-group ----
                nc.sync.dma_start(out[:, h0 : h0 + NH, :], t_t[:, :, :])

    nc.compile()
    return nc


def _get_nc():
    if "nc" not in _CACHE:
        _CACHE["nc"] = _build_nc()
    return _CACHE["nc"]


def kernel(x_1: np.ndarray, x_2: np.ndarray) -> np.ndarray:
    import ml_dtypes
    from concourse.bass_utils import run_bass_kernel_spmd

    nc = _get_nc()
    bf16 = ml_dtypes.bfloat16
    x_1 = np.ascontiguousarray(x_1.astype(bf16))
    x_2 = np.ascontiguousarray(x_2.astype(bf16))
    ident = np.eye(128, dtype=bf16)
    in_maps = [
        {"x1": x_1[b], "x2": x_2[b], "ident": ident} for b in range(B)
    ]
    res = run_bass_kernel_spmd(nc, in_maps, list(range(B)))
    return np.stack([res.results[b]["out"] for b in range(B)], axis=0)
